# revision 1
# baseline (speedup 1.0000x reference)
"""Trainium2 Bass kernel for the Brill-Lindquist Christoffel-symbol grid.

Math: the reference reduces to
    psi  = 1 + sum_n m_n / (2 r_n),   m = softplus(pre)
    h    = psi^4                       (= exp(4*log(psi)))
    G_c  = finite-difference gradient of h along grid axis c (2nd order
           central interior, 1st order one-sided edges, spacing DX)
    W_c  = 0.5 * G_c / h
    Gamma^i_{jk} = delta_ij W_k + delta_ik W_j - delta_jk W_i
so the [96,96,96,3,3,3] output is +-W_c scattered over 27 slots per point.

Sharding: axis 0 (12 planes per core x 8 cores). h is analytic in the
inputs, so each core evaluates its slab plus a 1-plane halo directly --
no inter-core exchange. Per core the grid is row-packed: row = a0*96+a1
(1152 rows -> 9 tiles of 128 partitions), free dim = a2 (96); h lives on
an 11-tile extended row window (halo tiles at both ends).

Axis-0/1 derivatives: SBUF->SBUF DMAs build row-shifted copies of the h
field (+-96 rows for axis 0, +-1 row for axis 1); the derivative is then
an aligned elementwise subtract. Grid-edge one-sided differences are
restored by rewriting the difference rows as 2*(h_shift - h) (ghost-cell
linear extrapolation), grouped into a few wide APs; the a0 edges exist
only on cores 0/7 and are applied through a per-core 0/1 mask with
copy_predicated. The axis-2 derivative uses shifted free-dim slices.

Output assembly: W is written (a2,c)-interleaved; the 9 diagonal slots
fill with one 4-dim-AP copy, the 12 off-diagonal nonzeros with 6
paired-slot ops (stride-0 source broadcast), zeros persist in reused
output buffers. Output DMA is contiguous per row (10368 B).
"""

import numpy as np

RES = 96
N_CORES = 8
PLANES = RES // N_CORES        # 12
LROWS = PLANES * RES           # 1152 local rows
NT = LROWS // 128              # 9 local 128-row tiles
EXTNT = NT + 2                 # 11 extended tiles (halo)
NROWS_G = RES * RES            # 9216 global rows
S27 = 27
NOB = 3                        # rotating output buffers

# Broadcast-row layout (free offsets in the [128, BCW] broadcast tile)
B_POS = 0        # px1,py1,pz1,px2,py2,pz2
B_S = 6          # m1/2, m2/2
B_CROW = 8       # (z - pz1)^2 [96], (z - pz2)^2 [96]
B_KVEC = 200     # z-FD column scale [96]
BCW = 296

# a1-edge fixup groups: (partition, first block, nblocks step 3)
A1LO_GROUPS = [(0, 0), (96, 0), (64, 1), (32, 2)]    # rows with a1 == 0
A1HI_GROUPS = [(95, 0), (63, 1), (31, 2), (127, 2)]  # rows with a1 == 95


def _grid_x():
    # Match the reference grid bit-for-bit: jnp.linspace in fp32 on CPU
    # (the reference's softplus cannot compile for the neuron backend, so
    # it necessarily runs on the jax CPU platform).
    import jax
    import jax.numpy as jnp
    MAX_X = 1.0
    DX = np.float32(MAX_X / (RES / 2 - 1))

    def _ls():
        return jnp.linspace(
            DX * (1 - RES / 2), DX * (RES / 2 - 1), RES, dtype=jnp.float32
        )

    try:
        with jax.default_device(jax.devices("cpu")[0]):
            x = np.asarray(_ls())
    except Exception:
        x = np.asarray(_ls())
    return x, float(DX)


def _fd_sources(idx, coeff_c, coeff_e):
    """(offset, coeff) pairs for d/didx with 1st-order one-sided edges."""
    if idx == 0:
        return [(1, coeff_e), (0, -coeff_e)]
    if idx == RES - 1:
        return [(0, coeff_e), (-1, -coeff_e)]
    return [(1, coeff_c), (-1, -coeff_c)]


def _build_dmat(core, DX):
    """[128, 6*3*128] bf16 FD matrices as matmul lhsT ([q, p] = coeff of
    ext-row q in output row p); 0.5 Christoffel factor folded in. All
    values are +-0.25/DX or +-0.5/DX = +-11.75 / +-23.5, exact in bf16.
    Entries: 0 g0(t=0), 1 g0(interior), 2 g0(t=8), 3..5 g1(t%3)."""
    import ml_dtypes
    c0 = 0.5 * (1.0 / (2.0 * np.float64(DX)))
    ce = 0.5 * (1.0 / np.float64(DX))
    out = np.zeros((128, 6 * 3 * 128), np.float64)

    def fill(entry, t, axis):
        for p in range(128):
            gr = core * LROWS + 128 * t + p
            a = (gr // RES) if axis == 0 else (gr % RES)
            step = RES if axis == 0 else 1
            for off, cf in _fd_sources(a, c0, ce):
                g2 = gr + off * step
                e_ = g2 - core * LROWS + 128
                j = e_ // 128 - t
                q = e_ - 128 * (t + j)
                assert 0 <= j <= 2 and 0 <= q < 128, (core, t, p, off)
                out[q, (entry * 3 + j) * 128 + p] = cf

    fill(0, 0, 0)
    fill(1, 1, 0)
    fill(2, NT - 1, 0)
    for v in range(3):
        fill(3 + v, v, 1)
    return out.astype(ml_dtypes.bfloat16)


def _build_static(core, x, DX):
    slab = core * LROWS
    e = np.arange(EXTNT * 128)
    g = np.clip(slab - 128 + e, 0, NROWS_G - 1)   # clamp halo overrun (unused rows)
    xcol = x[g % RES].reshape(EXTNT, 128).T.copy()     # X coordinate (a1)
    ycol = x[g // RES].reshape(EXTNT, 128).T.copy()    # Y coordinate (a0)
    kvec = np.full(RES, 0.25 / DX, np.float64)
    kvec[0] = kvec[-1] = 0.5 / DX
    return {
        "xcol": np.ascontiguousarray(xcol, np.float32),
        "ycol": np.ascontiguousarray(ycol, np.float32),
        "zrow": x.reshape(1, RES).astype(np.float32),
        "kvec": kvec.reshape(1, RES).astype(np.float32),
        "dmat": _build_dmat(core, DX),
    }


def _build_program(DX):
    import dataclasses as _dc

    import concourse.bacc as bacc
    import concourse.mybir as mybir
    import concourse.tile as tile

    DT = mybir.dt.float32
    BF = mybir.dt.bfloat16
    AF = mybir.ActivationFunctionType

    nc = bacc.Bacc(None, target_bir_lowering=False, debug=True)
    d_pos = nc.dram_tensor("bh_pos", [1, 6], DT, kind="ExternalInput")
    d_m = nc.dram_tensor("bh_m", [1, 2], DT, kind="ExternalInput")
    d_xcol = nc.dram_tensor("xcol", [128, EXTNT], DT, kind="ExternalInput")
    d_ycol = nc.dram_tensor("ycol", [128, EXTNT], DT, kind="ExternalInput")
    d_zrow = nc.dram_tensor("zrow", [1, RES], DT, kind="ExternalInput")
    d_kvec = nc.dram_tensor("kvec", [1, RES], DT, kind="ExternalInput")
    d_dmat = nc.dram_tensor("dmat", [128, 6 * 3 * 128], BF, kind="ExternalInput")
    d_out = nc.dram_tensor("out", [LROWS, RES * S27], DT, kind="ExternalOutput")

    HW_ = EXTNT * RES             # 1056: free width of the ext h field
    HCHUNKS = [(0, 3), (3, 6), (6, 9), (9, 11)]   # ext-block ranges
    with tile.TileContext(nc) as tc:
        with (
            tc.tile_pool(name="const", bufs=1) as cpool,
            tc.tile_pool(name="work", bufs=3) as wpool,
            tc.tile_pool(name="wout", bufs=3) as wopool,
            tc.tile_pool(name="obuf", bufs=NOB) as opool,
            tc.tile_pool(name="psum", bufs=2, space="PSUM") as pspool,
            tc.tile_pool(name="psb", bufs=1, space="PSUM") as psbpool,
        ):
            # --- constants in ---
            dm = cpool.tile([128, 6 * 3 * 128], BF)
            nc.sync.dma_start(dm[:], d_dmat[:])
            xc = cpool.tile([128, EXTNT], DT)
            nc.sync.dma_start(xc[:], d_xcol[:])
            yc = cpool.tile([128, EXTNT], DT)
            nc.sync.dma_start(yc[:], d_ycol[:])
            zr = cpool.tile([1, RES], DT)
            nc.sync.dma_start(zr[:], d_zrow[:])

            # --- broadcast row R: pos | m/2 | (z-pz)^2 x2 | kvec ---
            R = cpool.tile([1, BCW], DT)
            nc.sync.dma_start(R[:, B_POS:B_POS + 6], d_pos[:])
            nc.sync.dma_start(R[:, B_KVEC:B_KVEC + RES], d_kvec[:])
            m = cpool.tile([1, 2], DT)
            nc.sync.dma_start(m[:], d_m[:])
            nc.vector.tensor_scalar_mul(R[:, B_S:B_S + 2], m[:], 0.5)
            for n in range(2):
                dzn = cpool.tile([1, RES], DT, tag="dzn")
                nc.vector.tensor_scalar_sub(dzn[:], zr[:], R[:, 2 + 3 * n:3 + 3 * n])
                nc.vector.tensor_mul(
                    R[:, B_CROW + RES * n:B_CROW + RES * (n + 1)], dzn[:], dzn[:]
                )
            ones = cpool.tile([1, 128], DT)
            nc.vector.memset(ones[:], 1.0)
            bps = psbpool.tile([128, BCW], DT)
            nc.tensor.matmul(bps[:], ones[:], R[:])
            Bb = cpool.tile([128, BCW], DT)
            nc.vector.tensor_copy(Bb[:], bps[:])

            # --- per-partition (x-px)^2+(y-py)^2 for the 11 ext tiles ---
            ab = []
            for n in range(2):
                dxn = cpool.tile([128, EXTNT], DT, tag="dxn")
                nc.vector.tensor_scalar_sub(dxn[:], xc[:], Bb[:, 3 * n:3 * n + 1])
                dyn = cpool.tile([128, EXTNT], DT, tag="dyn")
                nc.vector.tensor_scalar_sub(dyn[:], yc[:], Bb[:, 3 * n + 1:3 * n + 2])
                nc.vector.tensor_mul(dxn[:], dxn[:], dxn[:])
                nc.vector.tensor_mul(dyn[:], dyn[:], dyn[:])
                abn = cpool.tile([128, EXTNT], DT, tag=f"ab{n}")
                nc.vector.tensor_add(abn[:], dxn[:], dyn[:])
                ab.append(abn)

            # --- h = psi^4 on the extended field + 3-way bf16 split ---
            # psi = 1 + (mh1*r2 + mh2*r1)/(r1*r2); h = ((psi)^2)^2
            # processed in 3-block-wide chunks to amortize per-op overhead
            H = cpool.tile([128, HW_], DT)
            Hh = cpool.tile([128, HW_], BF)
            Hm = cpool.tile([128, HW_], BF)
            Hl = cpool.tile([128, HW_], BF)
            for b0, b1 in HCHUNKS:
                nb = b1 - b0
                W = nb * RES
                csl = slice(RES * b0, RES * b1)
                rr = []
                for n in range(2):
                    r2 = wpool.tile([128, W], DT, tag="r2")
                    r2v = r2[:].rearrange("p (b z) -> p b z", z=RES)
                    crow = Bb[:, B_CROW + RES * n:B_CROW + RES * (n + 1)]
                    crow_b = _dc.replace(crow, ap=[crow.ap[0], [0, nb], [1, RES]])
                    absl = ab[n][:, b0:b1]
                    ab_b = _dc.replace(absl, ap=[absl.ap[0], [1, nb], [0, RES]])
                    nc.gpsimd.tensor_add(r2v[:, :, :], crow_b, ab_b)
                    rn = wpool.tile([128, W], DT, tag=f"rr{n}")
                    nc.scalar.activation(rn[:], r2[:], AF.Sqrt)
                    rr.append(rn)
                v = wpool.tile([128, W], DT, tag="v")
                nc.gpsimd.tensor_mul(v[:], rr[0][:], rr[1][:])
                u1 = wpool.tile([128, W], DT, tag="u1")
                nc.scalar.mul(u1[:], rr[1][:], Bb[:, B_S:B_S + 1])
                u2 = wpool.tile([128, W], DT, tag="u2")
                nc.scalar.mul(u2[:], rr[0][:], Bb[:, B_S + 1:B_S + 2])
                u = wpool.tile([128, W], DT, tag="u")
                nc.gpsimd.tensor_add(u[:], u1[:], u2[:])
                vinv = wpool.tile([128, W], DT, tag="vinv")
                vscr = wpool.tile([128, W], DT, tag="vscr")
                nc.vector.reciprocal_approx_accurate(vinv[:], v[:], vscr[:])
                psim = wpool.tile([128, W], DT, tag="psim")
                nc.vector.tensor_mul(psim[:], u[:], vinv[:])
                hsq = wpool.tile([128, W], DT, tag="hsq")
                nc.scalar.activation(hsq[:], psim[:], AF.Square, bias=1.0)
                nc.scalar.activation(H[:, csl], hsq[:], AF.Square)
                # 3-way bf16 split: h = hi + mid + lo (+ O(2^-27 h))
                nc.scalar.copy(Hh[:, csl], H[:, csl])
                s1 = wpool.tile([128, W], DT, tag="s1")
                nc.gpsimd.tensor_sub(s1[:], H[:, csl], Hh[:, csl])
                nc.scalar.copy(Hm[:, csl], s1[:])
                s2 = wpool.tile([128, W], DT, tag="s2")
                nc.gpsimd.tensor_sub(s2[:], s1[:], Hm[:, csl])
                nc.vector.tensor_copy(Hl[:, csl], s2[:])

            # --- rotating output buffers, zero slots pre-filled once ---
            otiles = []
            for i in range(NOB):
                O = opool.tile([128, RES * S27], DT, tag=f"ob{i}")
                O3 = O[:].rearrange("p (z s) -> p z s", s=S27)
                nc.gpsimd.memset(O3[:, :, 5:8:2], 0.0)
                nc.gpsimd.memset(O3[:, :, 11:20:4], 0.0)
                nc.gpsimd.memset(O3[:, :, 21], 0.0)
                otiles.append(O)

            # --- per local tile: FD matmuls, W, scatter, store ---
            for t in range(NT):
                g0e = 0 if t == 0 else (2 if t == NT - 1 else 1)
                g1e = 3 + (t % 3)
                hsl = slice(RES * (t + 1), RES * (t + 2))
                p0 = pspool.tile([128, RES], DT, tag="p0")
                p1 = pspool.tile([128, RES], DT, tag="p1")
                for ge, pp in ((g0e, p0), (g1e, p1)):
                    k = 0
                    for j in range(3):
                        lhs = dm[:, (ge * 3 + j) * 128:(ge * 3 + j + 1) * 128]
                        rsl = slice(RES * (t + j), RES * (t + j + 1))
                        for Hs in (Hh, Hm, Hl):
                            nc.tensor.matmul(
                                pp[:], lhs, Hs[:, rsl], start=(k == 0), stop=(k == 8)
                            )
                            k += 1

                hinv = wopool.tile([128, RES], DT, tag="hinv")
                nc.vector.reciprocal_approx_fast(hinv[:], H[:, hsl])
                hz = wopool.tile([128, RES], DT, tag="hz")
                nc.gpsimd.tensor_mul(hz[:], hinv[:], Bb[:, B_KVEC:B_KVEC + RES])

                w3 = wopool.tile([128, 3 * RES], DT, tag="w3")
                W3v = w3[:].rearrange("p (z c) -> p z c", c=3)
                nc.vector.tensor_mul(W3v[:, :, 0], p0[:], hinv[:])
                nc.vector.tensor_mul(W3v[:, :, 1], p1[:], hinv[:])
                st = wopool.tile([128, RES], DT, tag="st")
                nc.gpsimd.tensor_sub(st[:, 1:95], H[:, hsl][:, 2:96], H[:, hsl][:, 0:94])
                nc.gpsimd.tensor_sub(st[:, 0:1], H[:, hsl][:, 1:2], H[:, hsl][:, 0:1])
                nc.gpsimd.tensor_sub(
                    st[:, 95:96], H[:, hsl][:, 95:96], H[:, hsl][:, 94:95]
                )
                nc.vector.tensor_mul(W3v[:, :, 2], st[:], hz[:])

                O = otiles[t % NOB]
                O3 = O[:].rearrange("p (z s) -> p z s", s=S27)
                # 9 diagonal slots (i==j rows) in one op: slot a2*27+12i+c
                ddst = _dc.replace(
                    O[:], ap=[O[:].ap[0], [S27, RES], [12, 3], [1, 3]]
                )
                dsrc = _dc.replace(
                    w3[:], ap=[w3[:].ap[0], [3, RES], [0, 3], [1, 3]]
                )
                nc.scalar.copy(ddst, dsrc)
                # remaining 12 nonzero slots: 6 paired-slot ops
                for (a, b, c, sg) in (
                    (10, 20, 0, 1), (3, 23, 1, 1), (6, 16, 2, 1),
                    (4, 8, 0, -1), (9, 17, 1, -1), (18, 22, 2, -1),
                ):
                    dst = O3[:, :, a:b + 1:b - a]
                    src = _dc.replace(
                        W3v[:, :, c], ap=W3v[:, :, c].ap + [[0, 2]]
                    )
                    if sg > 0:
                        nc.vector.tensor_copy(dst, src)
                    else:
                        nc.vector.tensor_scalar_mul(dst, src, -1.0)
                nc.sync.dma_start(d_out[128 * t:128 * (t + 1), :], O[:])

    nc.finalize()
    return nc


_CACHE = {}


def _get_setup():
    if "nc" not in _CACHE:
        x, DX = _grid_x()
        _CACHE["static"] = [_build_static(c, x, DX) for c in range(N_CORES)]
        _CACHE["nc"] = _build_program(DX)
    return _CACHE["nc"], _CACHE["static"]


def kernel(BH_positions, BH_masses_presoftplus):
    from concourse.bass_utils import run_bass_kernel_spmd

    nc, static = _get_setup()
    pos = np.ascontiguousarray(np.asarray(BH_positions, np.float32).reshape(1, 6))
    # softplus of the two mass parameters (log1p(exp(x)) in fp32, as jax.nn.softplus)
    pre = np.asarray(BH_masses_presoftplus, np.float32)
    masses = np.log1p(np.exp(pre)).astype(np.float32).reshape(1, 2)
    in_maps = [{"bh_pos": pos, "bh_m": masses, **static[c]} for c in range(N_CORES)]
    res = run_bass_kernel_spmd(nc, in_maps, list(range(N_CORES)))
    parts = [
        res.results[c]["out"].reshape(PLANES, RES, RES, 3, 3, 3)
        for c in range(N_CORES)
    ]
    return np.ascontiguousarray(np.concatenate(parts, axis=0))



# revision 3
# speedup vs baseline: 1.3435x; 1.3435x over previous
"""Trainium2 Bass kernel for the Brill-Lindquist Christoffel-symbol grid.

Math: the reference reduces to
    psi  = 1 + sum_n m_n / (2 r_n),   m = softplus(pre)
    h    = psi^4
    G_c  = finite-difference gradient of h along grid axis c (2nd order
           central interior, 1st order one-sided edges, spacing DX)
    W_c  = 0.5 * G_c / h
    Gamma^i_{jk} = delta_ij W_k + delta_ik W_j - delta_jk W_i
so the [96,96,96,3,3,3] output is +-W_c scattered over 27 slots per point.

Sharding: axis 0 (12 planes per core x 8 cores). h is analytic in the
inputs, so each core evaluates its slab plus a 1-plane halo directly --
no inter-core exchange. Per core the grid is row-packed: row = a0*96+a1
(1152 rows -> 9 tiles of 128 partitions), free dim = a2 (96); h lives on
an 11-tile extended row window (halo tiles at both ends).

v2 design (output-DMA-roofline-targeted, ~12 MB/core at ~410 GB/s):
  - h chain via the fraction trick with HOST-prescaled squared radii:
    s_n = sqrt((r_n/mh_n)^2), psi-1 = (s1+s2)/(s1*s2), one DVE
    reciprocal_approx_fast, two Act squares. hsq=(psi)^2 kept fp32;
    H = hsq^2 stored bf16 only (matmul rhs + z-FD source).
  - axis-0/1 FD via 6 bf16 matmuls per tile (fp32 psum), single-bf16 h
    (measured rel err 2.9e-3 vs 2e-2 budget on the fixed seed).
  - scatter fused: psum x hinv written straight into the 27-slot
    interleaved output tile (no w3 intermediate); negated slots via
    scalar_tensor_tensor / Copy-with-scale=-1.
  - chunk/tile interleaved emission so tile t's chain starts as soon as
    its 3-block h window exists; 4 rotating output buffers; PE warmup
    matmuls + early Act-table preload to shorten the pipeline fill.
"""

import numpy as np

RES = 96
N_CORES = 8
PLANES = RES // N_CORES        # 12
LROWS = PLANES * RES           # 1152 local rows
NT = LROWS // 128              # 9 local 128-row tiles
EXTNT = NT + 2                 # 11 extended tiles (halo)
NROWS_G = RES * RES            # 9216 global rows
S27 = 27
NOB = 4                        # rotating output buffers

# misc input tile [128, MW] column layout
M_CROW = 0        # (z-pz1)^2/mh1^2 [96], (z-pz2)^2/mh2^2 [96]
M_KVEC = 192      # z-FD column scale [96] (0.25/DX interior, 0.5/DX edge)
M_AB = 288        # (x-px_n)^2+(y-py_n)^2)/mh_n^2 [11] x2
MW = 312

# dmat entry order: tile-0's two entries first so a small leading DMA
# unblocks the first tile's matmuls early.
ORDER = [0, 3, 1, 4, 5, 2]
SLOT = {e: i for i, e in enumerate(ORDER)}

# h-phase chunks (ext-block ranges): single blocks first for fill latency
CHUNKS = [(0, 1), (1, 2), (2, 3), (3, 6), (6, 9), (9, 11)]
# tiles emitted after the chunk that completes their 3-block window
TILES_AFTER_CHUNK = {2: [0], 3: [1, 2, 3], 4: [4, 5, 6], 5: [7, 8]}


def _grid_x():
    # Match the reference grid bit-for-bit: jnp.linspace in fp32 on CPU
    # (the reference's softplus cannot compile for the neuron backend, so
    # it necessarily runs on the jax CPU platform).
    import jax
    import jax.numpy as jnp
    MAX_X = 1.0
    DX = np.float32(MAX_X / (RES / 2 - 1))

    def _ls():
        return jnp.linspace(
            DX * (1 - RES / 2), DX * (RES / 2 - 1), RES, dtype=jnp.float32
        )

    try:
        with jax.default_device(jax.devices("cpu")[0]):
            x = np.asarray(_ls())
    except Exception:
        x = np.asarray(_ls())
    return x, float(DX)


def _fd_sources(idx, coeff_c, coeff_e):
    """(offset, coeff) pairs for d/didx with 1st-order one-sided edges."""
    if idx == 0:
        return [(1, coeff_e), (0, -coeff_e)]
    if idx == RES - 1:
        return [(0, coeff_e), (-1, -coeff_e)]
    return [(1, coeff_c), (-1, -coeff_c)]


def _build_dmat(core, DX):
    """[128, 6*3*128] bf16 FD matrices as matmul lhsT ([q, p] = coeff of
    ext-row q in output row p); 0.5 Christoffel factor folded in. All
    values are +-0.25/DX or +-0.5/DX = +-11.75 / +-23.5, exact in bf16.
    Logical entries: 0 g0(t=0), 1 g0(interior), 2 g0(t=8), 3..5 g1(t%3);
    stored in column slots per ORDER."""
    import ml_dtypes
    c0 = 0.5 * (1.0 / (2.0 * np.float64(DX)))
    ce = 0.5 * (1.0 / np.float64(DX))
    out = np.zeros((128, 6 * 3 * 128), np.float64)

    def fill(entry, t, axis):
        slot = SLOT[entry]
        for p in range(128):
            gr = core * LROWS + 128 * t + p
            a = (gr // RES) if axis == 0 else (gr % RES)
            step = RES if axis == 0 else 1
            for off, cf in _fd_sources(a, c0, ce):
                g2 = gr + off * step
                e_ = g2 - core * LROWS + 128
                j = e_ // 128 - t
                q = e_ - 128 * (t + j)
                assert 0 <= j <= 2 and 0 <= q < 128, (core, t, p, off)
                out[q, (slot * 3 + j) * 128 + p] = cf

    fill(0, 0, 0)
    fill(1, 1, 0)
    fill(2, NT - 1, 0)
    for v in range(3):
        fill(3 + v, v, 1)
    return out.astype(ml_dtypes.bfloat16)


def _g0_slot(t):
    return SLOT[0] if t == 0 else (SLOT[2] if t == NT - 1 else SLOT[1])


def _g1_slot(t):
    return SLOT[3 + (t % 3)]


def _build_static(core, x, DX):
    slab = core * LROWS
    e = np.arange(EXTNT * 128)
    g = np.clip(slab - 128 + e, 0, NROWS_G - 1)   # clamp halo overrun (unused rows)
    xcol = x[g % RES].reshape(EXTNT, 128).T.copy()     # X coordinate (a1)
    ycol = x[g // RES].reshape(EXTNT, 128).T.copy()    # Y coordinate (a0)
    kvec = np.full(RES, 0.25 / DX, np.float64)
    kvec[0] = kvec[-1] = 0.5 / DX
    return {
        "xcol": np.ascontiguousarray(xcol, np.float64),
        "ycol": np.ascontiguousarray(ycol, np.float64),
        "kvec": kvec,
        "dmat": _build_dmat(core, DX),
    }


def _build_misc(static, x, pos, mh):
    """Per-core [128, MW] fp32 misc tile: prescaled crow/ab + kvec."""
    misc = np.zeros((128, MW), np.float64)
    for n in range(2):
        crow = (x.astype(np.float64) - pos[n, 2]) ** 2 / (mh[n] * mh[n])
        misc[:, M_CROW + RES * n:M_CROW + RES * (n + 1)] = crow[None, :]
        ab = (static["xcol"] - pos[n, 0]) ** 2 + (static["ycol"] - pos[n, 1]) ** 2
        misc[:, M_AB + EXTNT * n:M_AB + EXTNT * (n + 1)] = ab / (mh[n] * mh[n])
    misc[:, M_KVEC:M_KVEC + RES] = static["kvec"][None, :]
    return np.ascontiguousarray(misc, np.float32)


def _build_program():
    import dataclasses as _dc

    import concourse.bacc as bacc
    import concourse.mybir as mybir
    import concourse.tile as tile

    DT = mybir.dt.float32
    BF = mybir.dt.bfloat16
    AF = mybir.ActivationFunctionType
    MUL = mybir.AluOpType.mult

    nc = bacc.Bacc(None, target_bir_lowering=False, debug=True)
    d_misc = nc.dram_tensor("misc", [128, MW], DT, kind="ExternalInput")
    d_dmat = nc.dram_tensor("dmat", [128, 6 * 3 * 128], BF, kind="ExternalInput")
    d_out = nc.dram_tensor("out", [LROWS, RES * S27], DT, kind="ExternalOutput")

    HW_ = EXTNT * RES             # 1056: free width of the ext h field
    with tile.TileContext(nc) as tc:
        with (
            tc.tile_pool(name="const", bufs=1) as cpool,
            tc.tile_pool(name="work", bufs=3) as wpool,
            tc.tile_pool(name="wout", bufs=3) as wopool,
            tc.tile_pool(name="obuf", bufs=NOB) as opool,
            tc.tile_pool(name="psum", bufs=3, space="PSUM") as pspool,
            tc.tile_pool(name="psw", bufs=1, space="PSUM") as pswpool,
        ):
            # --- inputs in: misc first (unblocks h chain), dmat in two
            # pieces (tile-0's entries lead) ---
            mi = cpool.tile([128, MW], DT)
            nc.sync.dma_start(mi[:], d_misc[:])
            dm = cpool.tile([128, 6 * 3 * 128], BF)
            nc.sync.dma_start(dm[:, :6 * 128], d_dmat[:, :6 * 128])
            nc.sync.dma_start(dm[:, 6 * 128:], d_dmat[:, 6 * 128:])

            # --- warmups: Act table preload (sqrt set) + PE pstate ramp ---
            junk = cpool.tile([128, 256], BF, tag="junk")
            nc.vector.memset(junk[:], 1.0)
            jact = cpool.tile([1, 8], DT, tag="jact")
            nc.scalar.activation(jact[:], junk[0:1, 0:8], AF.Sqrt)
            jps = pswpool.tile([128, 256], DT)
            for _ in range(8):
                nc.tensor.matmul(
                    jps[:], junk[:, :128], junk[:], start=True, stop=True
                )

            # --- persistent fields ---
            HSQ = cpool.tile([128, HW_], DT)   # psi^2, fp32 (hinv source)
            Hb = cpool.tile([128, HW_], BF)    # h = psi^4, bf16 (FD source)

            # --- rotating output buffers, zero slots pre-filled once ---
            otiles = []
            for i in range(NOB):
                O = opool.tile([128, RES * S27], DT, tag=f"ob{i}")
                O3 = O[:].rearrange("p (z s) -> p z s", s=S27)
                nc.gpsimd.memset(O3[:, :, 5:8:2], 0.0)
                nc.gpsimd.memset(O3[:, :, 11:20:4], 0.0)
                nc.gpsimd.memset(O3[:, :, 21], 0.0)
                otiles.append(O)

            def h_chunk(b0, b1):
                nb = b1 - b0
                W = nb * RES
                csl = slice(RES * b0, RES * b1)
                ss = []
                for n in range(2):
                    r2 = wpool.tile([128, W], DT, tag="r2")
                    r2v = r2[:].rearrange("p (b z) -> p b z", z=RES)
                    crow = mi[:, M_CROW + RES * n:M_CROW + RES * (n + 1)]
                    crow_b = _dc.replace(
                        crow, ap=[crow.ap[0], [0, nb], [1, RES]]
                    )
                    absl = mi[:, M_AB + EXTNT * n + b0:M_AB + EXTNT * n + b1]
                    ab_b = _dc.replace(
                        absl, ap=[absl.ap[0], [1, nb], [0, RES]]
                    )
                    nc.vector.tensor_add(r2v[:, :, :], crow_b, ab_b)
                    sn = wpool.tile([128, W], DT, tag=f"s{n}")
                    nc.scalar.activation(sn[:], r2[:], AF.Sqrt)
                    ss.append(sn)
                v = wpool.tile([128, W], DT, tag="v")
                nc.gpsimd.tensor_mul(v[:], ss[0][:], ss[1][:])
                u = wpool.tile([128, W], DT, tag="u")
                nc.gpsimd.tensor_add(u[:], ss[0][:], ss[1][:])
                vinv = wpool.tile([128, W], DT, tag="vinv")
                nc.vector.reciprocal_approx_fast(vinv[:], v[:])
                psim = wpool.tile([128, W], DT, tag="psim")
                nc.vector.tensor_mul(psim[:], u[:], vinv[:])
                nc.scalar.activation(HSQ[:, csl], psim[:], AF.Square, bias=1.0)
                nc.scalar.activation(Hb[:, csl], HSQ[:, csl], AF.Square)

            def do_tile(t):
                hsl = slice(RES * (t + 1), RES * (t + 2))
                p0 = pspool.tile([128, RES], DT, tag="p0")
                p1 = pspool.tile([128, RES], DT, tag="p1")
                for slot, pp in ((_g0_slot(t), p0), (_g1_slot(t), p1)):
                    for j in range(3):
                        lhs = dm[:, (slot * 3 + j) * 128:(slot * 3 + j + 1) * 128]
                        rsl = slice(RES * (t + j), RES * (t + j + 1))
                        nc.tensor.matmul(
                            pp[:], lhs, Hb[:, rsl], start=(j == 0), stop=(j == 2)
                        )

                q = wopool.tile([128, RES], DT, tag="q")
                nc.vector.reciprocal_approx_fast(q[:], HSQ[:, hsl])
                hinv = wopool.tile([128, RES], DT, tag="hinv")
                nc.scalar.activation(hinv[:], q[:], AF.Square)
                hz = wopool.tile([128, RES], DT, tag="hz")
                nc.gpsimd.tensor_mul(hz[:], hinv[:], mi[:, M_KVEC:M_KVEC + RES])
                st = wopool.tile([128, RES], DT, tag="st")
                Ht = Hb[:, hsl]
                nc.gpsimd.tensor_sub(st[:, 1:95], Ht[:, 2:96], Ht[:, 0:94])
                nc.gpsimd.tensor_sub(st[:, 0:1], Ht[:, 1:2], Ht[:, 0:1])
                nc.gpsimd.tensor_sub(st[:, 95:96], Ht[:, 95:96], Ht[:, 94:95])
                w2 = wopool.tile([128, RES], DT, tag="w2")
                nc.gpsimd.tensor_mul(w2[:], st[:], hz[:])

                O = otiles[t % NOB]
                O3 = O[:].rearrange("p (z s) -> p z s", s=S27)

                def bcast(ap_, k):
                    return _dc.replace(ap_, ap=ap_.ap + [[0, k]])

                # c=0 (+W0 at {0,12,24},{10,20}; -W0 at {4,8}) from p0*hinv
                # c=1 (+W1 at {1,13,25},{3,23}; -W1 at {9,17}) from p1*hinv
                for pp, trio, pair, neg in (
                    (p0, O3[:, :, 0:25:12], O3[:, :, 10:21:10], O3[:, :, 4:9:4]),
                    (p1, O3[:, :, 1:26:12], O3[:, :, 3:24:20], O3[:, :, 9:18:8]),
                ):
                    nc.vector.tensor_mul(trio, bcast(pp[:], 3), bcast(hinv[:], 3))
                    nc.vector.tensor_mul(pair, bcast(pp[:], 2), bcast(hinv[:], 2))
                    nc.vector.scalar_tensor_tensor(
                        neg, bcast(pp[:], 2), -1.0, bcast(hinv[:], 2), MUL, MUL
                    )
                # c=2: +w2 at {2,14,26},{6,16}; -w2 at {18,22}
                nc.scalar.copy(O3[:, :, 2:27:12], bcast(w2[:], 3))
                nc.scalar.copy(O3[:, :, 6:17:10], bcast(w2[:], 2))
                nc.scalar.mul(O3[:, :, 18:23:4], bcast(w2[:], 2), -1.0)

                nc.sync.dma_start(d_out[128 * t:128 * (t + 1), :], O[:])

            for ci, (b0, b1) in enumerate(CHUNKS):
                h_chunk(b0, b1)
                for t in TILES_AFTER_CHUNK.get(ci, []):
                    do_tile(t)

    nc.finalize()
    return nc


_CACHE = {}


def _get_setup():
    if "nc" not in _CACHE:
        x, DX = _grid_x()
        _CACHE["x"] = x
        _CACHE["static"] = [_build_static(c, x, DX) for c in range(N_CORES)]
        _CACHE["nc"] = _build_program()
    return _CACHE["nc"], _CACHE["static"]


def _in_maps(BH_positions, BH_masses_presoftplus):
    nc, static = _get_setup()
    x = _CACHE["x"]
    pos = np.asarray(BH_positions, np.float64)
    pre = np.asarray(BH_masses_presoftplus, np.float64)
    mh = np.log1p(np.exp(pre)) * 0.5          # softplus(pre) / 2
    return [
        {"misc": _build_misc(static[c], x, pos, mh), "dmat": static[c]["dmat"]}
        for c in range(N_CORES)
    ]


def kernel(BH_positions, BH_masses_presoftplus):
    from concourse.bass_utils import run_bass_kernel_spmd

    nc, _ = _get_setup()
    in_maps = _in_maps(BH_positions, BH_masses_presoftplus)
    res = run_bass_kernel_spmd(nc, in_maps, list(range(N_CORES)))
    parts = [
        res.results[c]["out"].reshape(PLANES, RES, RES, 3, 3, 3)
        for c in range(N_CORES)
    ]
    return np.ascontiguousarray(np.concatenate(parts, axis=0))
